# revision 12
# baseline (speedup 1.0000x reference)
"""Gabor-modulated conv-weight synthesis on 8 Trainium2 NeuronCores.

Computes out[g*CO + co, ci, h, w] = gabor(theta[g], lam[g])[h, w] * x[co, ci, h, w]
for x: [512, 512, 9, 9] f32, theta/lam: [4] f32  ->  out: [2048, 512, 9, 9] f32.

Sharding: x along C_out into 8 shards of 64; theta/lam replicated; each core
produces its [4, 64, 512, 9, 9] output slice with no communication.

I/O precision: the harness gate is rel_err < 2e-2; fp16 I/O (worst-case rel
error ~7e-4 end to end) halves HBM traffic vs f32: 5.3 MB in + 21.2 MB out
per core = 26.5 MB, a ~62 us floor at the ~430 GB/s measured per-core DMA
rate. The host converts x f32->fp16 before upload and upcasts the result.

Schedule (all DMA on the two HWDGE rings; gpsimd SWDGE costs ~2 us per op
on the Q7 and is not used):
  - SP ring carries the consts load then the four x-chunk loads, in order:
    a ring is FIFO, so L0 completes ~3 us after the consts instead of
    round-robinning to completion alongside L1-L3 (which starved the first
    multiply until all loads finished),
  - consts (grids, envelope, sqrt(2*env), theta, lam) come pre-replicated
    to 128 partitions from the host (one dense load, no gpsimd
    partition_broadcast, which cost ~10 us of Q7 time),
  - Gabor synthesis in f32, minimized serial chain: cos th = Sin(-th+pi/2)
    on ACT, lam folded into the per-g grid coefficients, periodic range
    reduction via int32 round-trip, envelope applied as
    env*(1-2 ss^2) = env - (ss*sqrt(2 env))^2 with the fp16 cast fused
    into the final subtract,
  - multiply each x chunk by each filter on the DVE (free-dim-broadcast
    AP, fp16 2x mode, ~2.9 us per [128, 5184] tile); chunk-0 stores go to
    the ACT ring (SP is still streaming loads), later chunks alternate
    rings; the very last mul+store is split in half across both rings to
    shorten the end-of-kernel drain.
"""

import numpy as np

import concourse.bass as bass
import concourse.bacc as bacc
import concourse.mybir as mybir
from concourse.tile import TileContext
from concourse.bass_utils import run_bass_kernel_spmd

N_CORES = 8
G = 4
CO, CI, H, W = 512, 512, 9, 9
HW = H * W                # 81
CO_SH = CO // N_CORES     # 64 C_out rows per core
ROWS = CO_SH * CI         # 32768 (co_local, ci) rows per core
P = 128                   # SBUF partitions
NPP = ROWS // P           # 256 rows per partition
N_SUB = 64                # rows-per-partition per chunk
N_CHUNKS = NPP // N_SUB   # 4
SIGMA = float(np.pi)      # Gaussian envelope std of the Gabor synthesis
NCST = 4 * HW + 2 * G + 2  # x-grid, y-grid, env, sqrt(2*env), theta, lam, -1, pi/2

F32 = mybir.dt.float32
F16 = mybir.dt.float16
AF = mybir.ActivationFunctionType
ALU = mybir.AluOpType


def build_bass(rows=ROWS, n_sub=N_SUB):
    npp = rows // P
    n_chunks = npp // n_sub
    assert npp % n_sub == 0

    nc = bacc.Bacc("TRN2", target_bir_lowering=False, debug=False)
    x = nc.declare_dram_parameter("x", [rows, HW], F16, isOutput=False)
    cst = nc.declare_dram_parameter("cst", [P, NCST], F32, isOutput=False)
    out = nc.declare_dram_parameter("out", [G, rows, HW], F16, isOutput=True)

    xv = x.ap().rearrange("(p n) m -> p n m", p=P)                 # [128, npp, 81]
    ov = out.ap().rearrange("g (p n) m -> g p n m", p=P).transpose([1, 0, 2, 3])

    GHW = G * HW
    with TileContext(nc) as tc:
        with tc.tile_pool(name="consts", bufs=1) as cpool, \
             tc.tile_pool(name="xs", bufs=n_chunks) as xpool, \
             tc.tile_pool(name="outs", bufs=8) as opool:
            # consts + all x-chunk loads on the SP ring, in order (FIFO)
            allb = cpool.tile([P, NCST], F32)
            nc.sync.dma_start(allb, cst.ap())
            xtiles = []
            for i in range(n_chunks):
                xt = xpool.tile([P, n_sub * HW], F16, tag="x", name=f"xt{i}")
                nc.sync.dma_start(xt, xv[:, i * n_sub:(i + 1) * n_sub, :])
                xtiles.append(xt)

            cst_t = allb[:, 0:3 * HW]
            evh_t = allb[:, 3 * HW:4 * HW]
            th_t = allb[:, 4 * HW:4 * HW + G]
            lm_t = allb[:, 4 * HW + G:4 * HW + 2 * G]
            neg1 = allb[:, 4 * HW + 2 * G:4 * HW + 2 * G + 1]      # [P,1] -1.0
            hpi = allb[:, 4 * HW + 2 * G + 1:4 * HW + 2 * G + 2]   # [P,1] pi/2

            def per_g(t):  # [128, G] -> [128, G, HW] step-0 view
                return t[:, :].unsqueeze(2).broadcast_to([P, G, HW])

            def over_g(ap):  # [128, 81] -> [128, G, 81] step-0 view
                return ap.unsqueeze(1).broadcast_to([P, G, HW])

            xs_b = over_g(cst_t[:, 0:HW])
            ys_b = over_g(cst_t[:, HW:2 * HW])
            env_b = over_g(cst_t[:, 2 * HW:3 * HW])
            evh_b = over_g(evh_t)

            # sin/cos of theta on ACT (cos th = sin(pi/2 - th); th in [0, pi)
            # so the argument stays within ACT Sin's [-pi, pi] domain)
            sin_t = cpool.tile([P, G], F32)
            nc.scalar.activation(sin_t, th_t, AF.Sin)
            cos_t = cpool.tile([P, G], F32)
            nc.scalar.activation(cos_t, th_t, AF.Sin, bias=hpi, scale=neg1)
            # fold lam in: t = lam*(x cos th + y sin th) = x*cl + y*sl
            cl = cpool.tile([P, G], F32)
            nc.vector.tensor_mul(cl, cos_t, lm_t)
            sl = cpool.tile([P, G], F32)
            nc.vector.tensor_mul(sl, sin_t, lm_t)

            tt = cpool.tile([P, G, HW], F32)
            t2 = cpool.tile([P, G, HW], F32)
            nc.vector.tensor_mul(tt, xs_b, per_g(cl))
            nc.vector.tensor_mul(t2, ys_b, per_g(sl))
            nc.vector.tensor_add(tt, tt, t2)
            # range-reduce t to (-1, 1) via int32 round-trip (ACT Sin is only
            # valid on [-pi, pi]; DVE tensor_scalar has no mod). Any integer
            # shift k preserves cos(2pi t) = 1 - 2 sin^2(pi (t - k)).
            ti = cpool.tile([P, G, HW], mybir.dt.int32)
            nc.vector.tensor_copy(ti, tt)
            tf = cpool.tile([P, G, HW], F32)
            nc.vector.tensor_copy(tf, ti)
            nc.vector.tensor_sub(tt, tt, tf)
            ss = cpool.tile([P, G, HW], F32)
            nc.scalar.activation(ss, tt, AF.Sin, scale=SIGMA)      # sin(pi m)
            # gabor = env*cos(2pi t) = env*(1-2 ss^2) = env - (ss*sqrt(2 env))^2
            u = cpool.tile([P, G, HW], F32)
            nc.vector.tensor_mul(u, ss, evh_b)
            nc.vector.tensor_mul(u, u, u)
            gb16 = cpool.tile([P, GHW], F16)
            gbg16 = gb16.rearrange("p (g m) -> p g m", m=HW)
            nc.vector.tensor_tensor(gbg16, env_b, u, ALU.subtract)

            gbv = [
                gb16[:, g * HW:(g + 1) * HW].unsqueeze(1)
                .broadcast_to([P, n_sub, HW])
                for g in range(G)
            ]

            # ---- streaming broadcast-multiply ----
            # ring balance: chunk 0's first two stores ride the ACT ring (the
            # SP ring is still draining x loads); afterwards stores alternate,
            # ending with 8 x 1.33 MB on each ring. The final four stores are
            # split into 0.66 MB halves alternating rings so both rings drain
            # their backlog to within one half-tile of each other.
            for i in range(n_chunks):
                n0 = i * n_sub
                xtv = xtiles[i].rearrange("p (n m) -> p n m", m=HW)
                for g in range(G):
                    ot = opool.tile([P, n_sub * HW], F16, tag="o")
                    otv = ot.rearrange("p (n m) -> p n m", m=HW)
                    if i == 0:
                        eng = nc.scalar if g < 2 else nc.sync
                    else:
                        eng = nc.sync if g % 2 == 0 else nc.scalar
                    if i == n_chunks - 1 and g >= 2:
                        # halve the trailing muls+stores: finer drain granularity
                        half = n_sub // 2
                        for k in range(2):
                            nl, nh = k * half, (k + 1) * half
                            nc.vector.tensor_tensor(
                                otv[:, nl:nh], xtv[:, nl:nh],
                                gbv[g][:, nl:nh], ALU.mult,
                            )
                            eng2 = nc.sync if (g + k) % 2 == 0 else nc.scalar
                            eng2.dma_start(
                                ov[:, g, n0 + nl:n0 + nh, :], otv[:, nl:nh]
                            )
                    else:
                        nc.vector.tensor_tensor(otv, xtv, gbv[g], ALU.mult)
                        eng.dma_start(ov[:, g, n0:n0 + n_sub, :], otv)
    nc.finalize()  # Bacc passes: wait legalization, reg alloc, act table loads
    return nc


def make_consts():
    ys = np.arange(H, dtype=np.float32) - (H - 1) / 2.0
    xs = np.arange(W, dtype=np.float32) - (W - 1) / 2.0
    y, x = np.meshgrid(ys, xs, indexing="ij")
    env = np.exp(-(x ** 2 + y ** 2) / (2.0 * np.float32(SIGMA) ** 2))
    envh = np.sqrt(2.0 * env)
    return np.concatenate(
        [v.reshape(-1) for v in (x, y, env, envh)]
    ).astype(np.float32)  # [4 * 81]


def make_row(theta, lam):
    return np.concatenate([
        make_consts(), theta, lam,
        np.array([-1.0, np.pi / 2], dtype=np.float32),
    ])  # [NCST] f32


_NC = None
TRACE = False          # set True by the local test harness for NTFF timing
LAST_RESULT = None     # BassKernelResults of the most recent run


def kernel(x, theta, lam):
    global _NC
    if _NC is None:
        _NC = build_bass()
    x16 = np.ascontiguousarray(np.asarray(x)).astype(np.float16)
    theta = np.asarray(theta, dtype=np.float32).reshape(G)
    lam = np.asarray(lam, dtype=np.float32).reshape(G)
    row = make_row(theta, lam)                             # [NCST] f32
    cst = np.ascontiguousarray(np.broadcast_to(row, (P, NCST)))

    in_maps = []
    for m in range(N_CORES):
        shard = x16[m * CO_SH:(m + 1) * CO_SH].reshape(ROWS, HW)
        in_maps.append({"x": shard, "cst": cst})

    global LAST_RESULT
    LAST_RESULT = run_bass_kernel_spmd(
        _NC, in_maps, list(range(N_CORES)), trace=TRACE
    )
    res = LAST_RESULT.results

    out = np.empty((G, CO, CI, H, W), dtype=np.float32)
    for m in range(N_CORES):
        out[:, m * CO_SH:(m + 1) * CO_SH] = res[m]["out"].reshape(
            G, CO_SH, CI, H, W
        )
    return out.reshape(G * CO, CI, H, W)
